# revision 26
# baseline (speedup 1.0000x reference)
"""Trainium2 Bass kernel for nn_AttentionInterRNN (bidirectional GRU + attention pooling).

Strategy: data-parallel over batch B=64 across 8 NeuronCores (b=8 per core).
Per core, fully local (no collectives):
  Phase 1: xg = x @ w_ih.T + bias, computed gate-major (features on partitions)
           as fp32r matmuls, staged to DRAM.
  Phase 2: fwd+bwd GRU recurrences, interleaved per step. Recurrent matmul is
           gate-major with bf16 weights/h (fp32 PSUM accumulate, fp32 master h).
  Phase 3: attention pooling: squish = tanh(out @ W_inter + b), attn = squish @ proj,
           final = sum_t attn*(out @ W_final.T) + b_final. All fp32r matmuls.

All layout transposes are done host-side in numpy. Feature-major convention:
feature f lives at [partition p = f % 128, chunk c = f // 128].
"""
import os
import numpy as np
import ml_dtypes

import concourse.bass as bass
import concourse.tile as tile
from concourse.tile import add_dep_helper
from concourse import bacc, mybir
from concourse import bass_utils

T, B, I, H, C = 256, 64, 1024, 512, 32
NCORES = 8
b = B // NCORES          # 8 batch elements per core
TB = T * b               # 2048 flattened (t, b) per core
KI = I // 128            # 8  input chunks
KH = H // 128            # 4  hidden chunks
MG = 3 * H // 128        # 12 gate chunks
MJ = 2 * H // 128        # 8  chunks of 2H
NT = TB // 512           # 4  (t,b) chunks of 512

F32 = mybir.dt.float32
F32R = mybir.dt.float32r
BF16 = mybir.dt.bfloat16
NPBF16 = ml_dtypes.bfloat16

TRACE = os.environ.get("GRU_KERNEL_TRACE", "0") == "1"

_CACHE = {}


# ----------------------------------------------------------------- host prep

def _prep_shared(inp):
    """Weight tensors, identical on every core."""
    m = {}
    for d, suf in (("f", "_f"), ("b", "_b")):
        w_ih = np.asarray(inp["w_ih" + suf], np.float32)   # [3H, I]
        w_hh = np.asarray(inp["w_hh" + suf], np.float32)   # [3H, H]
        b_ih = np.asarray(inp["b_ih" + suf], np.float32)
        b_hh = np.asarray(inp["b_hh" + suf], np.float32)
        m["wih_" + d] = np.ascontiguousarray(
            w_ih.T.reshape(KI, 128, MG, 128).transpose(1, 0, 2, 3))
        m["whh_" + d] = np.ascontiguousarray(
            w_hh.T.reshape(KH, 128, MG, 128).transpose(1, 0, 2, 3)).astype(NPBF16)
        xb = b_ih.copy()
        xb[:2 * H] += b_hh[:2 * H]
        m["xbias_" + d] = np.ascontiguousarray(xb.reshape(MG, 128).T)
        m["hbias_" + d] = np.ascontiguousarray(
            b_hh[2 * H:].reshape(KH, 128)).astype(NPBF16)   # [4, 128] lhsT for bias MM
    m["onehot"] = np.kron(np.eye(KH, dtype=np.float32),
                          np.ones((1, b), np.float32)).astype(NPBF16)  # [4, 4*b]
    W_inter = np.asarray(inp["W_inter"], np.float32)
    m["winter"] = np.ascontiguousarray(
        W_inter.reshape(MJ, 128, MJ, 128).transpose(1, 0, 2, 3))
    m["bias_inter"] = np.ascontiguousarray(
        np.asarray(inp["bias_inter"], np.float32)[:, 0].reshape(MJ, 128).T)
    m["wproj"] = np.ascontiguousarray(
        np.asarray(inp["weight_proj"], np.float32)[:, 0].reshape(MJ, 128).T)
    m["wfinalT"] = np.ascontiguousarray(
        np.asarray(inp["W_final"], np.float32).T.reshape(MJ, 128, C).transpose(1, 0, 2))
    m["bfinal"] = np.asarray(inp["b_final"], np.float32).reshape(C, 1).copy()
    return m


def _prep_core(inp, ci):
    """Per-core input shards."""
    m = {}
    x = np.asarray(inp["intra_attention_vectors"], np.float32)[:, ci * b:(ci + 1) * b, :]
    m["xT"] = np.ascontiguousarray(
        x.transpose(2, 0, 1).reshape(KI, 128, TB).transpose(1, 0, 2))
    st = np.asarray(inp["state_inter"], np.float32)[:, ci * b:(ci + 1) * b, :]
    for di, d in enumerate(("f", "b")):
        m["h0_" + d] = np.ascontiguousarray(
            st[di].T.reshape(KH, 128, b).transpose(1, 0, 2))
    return m


# ----------------------------------------------------------------- kernel build

def _build():
    nc = bacc.Bacc("TRN2", target_bir_lowering=False, debug=False)

    # inputs
    xT = nc.dram_tensor("xT", (128, KI, TB), F32R, kind="ExternalInput").ap()
    wih = {d: nc.dram_tensor(f"wih_{d}", (128, KI, MG, 128), F32R, kind="ExternalInput").ap()
           for d in ("f", "b")}
    whh = {d: nc.dram_tensor(f"whh_{d}", (128, KH, MG, 128), BF16, kind="ExternalInput").ap()
           for d in ("f", "b")}
    h0 = {d: nc.dram_tensor(f"h0_{d}", (128, KH, b), F32, kind="ExternalInput").ap()
          for d in ("f", "b")}
    xbias = {d: nc.dram_tensor(f"xbias_{d}", (128, MG), F32, kind="ExternalInput").ap()
             for d in ("f", "b")}
    hbias = {d: nc.dram_tensor(f"hbias_{d}", (KH, 128), BF16, kind="ExternalInput").ap()
             for d in ("f", "b")}
    onehot = nc.dram_tensor("onehot", (KH, KH * b), BF16, kind="ExternalInput").ap()
    winter = nc.dram_tensor("winter", (128, MJ, MJ, 128), F32R, kind="ExternalInput").ap()
    bias_inter = nc.dram_tensor("bias_inter", (128, MJ), F32, kind="ExternalInput").ap()
    wproj = nc.dram_tensor("wproj", (128, MJ), F32R, kind="ExternalInput").ap()
    wfinalT = nc.dram_tensor("wfinalT", (128, MJ, C), F32R, kind="ExternalInput").ap()
    bfinal = nc.dram_tensor("bfinal", (C, 1), F32, kind="ExternalInput").ap()

    # outputs
    o_final = nc.dram_tensor("o_final", (C, b), F32, kind="ExternalOutput").ap()
    o_state = {d: nc.dram_tensor(f"o_state_{d}", (128, KH, b), F32, kind="ExternalOutput").ap()
               for d in ("f", "b")}
    o_attn = nc.dram_tensor("o_attn", (1, TB), F32, kind="ExternalOutput").ap()

    Ident = mybir.ActivationFunctionType.Identity
    Sig = mybir.ActivationFunctionType.Sigmoid
    Tanh = mybir.ActivationFunctionType.Tanh

    with tile.TileContext(nc) as tc:
        with tc.tile_pool(name="dram", bufs=1, space="DRAM") as dpool:
            xgc = {d: [dpool.tile([128, MG, 512], F32, name=f"xg_{d}_{n}")
                       for n in range(NT)] for d in ("f", "b")}
            ybuf = {d: dpool.tile([128, KH, TB], F32, name=f"ybuf_{d}") for d in ("f", "b")}
            attn_d = dpool.tile([1, TB], F32, name="attn_d")

            # ---------------- Phase 1: xg = w_ihT.T @ xT (+ bias) ----------
            with tc.tile_pool(name="p1w", bufs=1) as p1w, \
                 tc.tile_pool(name="p1x", bufs=1) as p1x, \
                 tc.tile_pool(name="p1s", bufs=6) as p1s, \
                 tc.tile_pool(name="p1ps", bufs=8, space="PSUM") as p1ps:
                xt = p1x.tile([128, KI, TB], F32R, name="xt")
                nc.sync.dma_start(out=xt[:], in_=xT[:])
                wt, xb = {}, {}
                for d in ("f", "b"):
                    wt[d] = p1w.tile([128, KI, MG, 128], F32R, tag=f"wih_{d}", name=f"wih_{d}")
                    nc.sync.dma_start(out=wt[d][:], in_=wih[d][:])
                    xb[d] = p1s.tile([128, MG], F32, tag=f"xbias_{d}", name=f"xb_{d}")
                    nc.sync.dma_start(out=xb[d][:], in_=xbias[d][:])
                for d, n in [("f", 0), ("b", 3), ("f", 1), ("b", 2),
                             ("f", 2), ("b", 1), ("f", 3), ("b", 0)]:
                    for m in range(MG):
                        ps1 = p1ps.tile([128, 512], F32, tag="ps", name=f"ps_{d}_{n}_{m}")
                        for k in range(KI):
                            nc.tensor.matmul(
                                ps1[:], lhsT=wt[d][:, k, m, :],
                                rhs=xt[:, k, n * 512:(n + 1) * 512],
                                start=(k == 0), stop=(k == KI - 1))
                        stg = p1s.tile([128, 512], F32, tag="stage", name=f"stg_{d}_{n}_{m}")
                        nc.scalar.activation(stg[:], ps1[:], Ident, bias=xb[d][:, m:m + 1])
                        nc.sync.dma_start(out=xgc[d][n][:, m, :], in_=stg[:])

            # ---------------- Phase 2: GRU recurrences (fwd+bwd interleaved)
            with tc.tile_pool(name="p2w", bufs=1) as p2w, \
                 tc.tile_pool(name="p2st", bufs=1) as p2st, \
                 tc.tile_pool(name="p2xg", bufs=3) as p2xg, \
                 tc.tile_pool(name="p2y", bufs=2) as p2y, \
                 tc.tile_pool(name="p2g", bufs=2) as p2g, \
                 tc.tile_pool(name="p2ps", bufs=2, space="PSUM") as p2ps:
                wsb, hbf, hbmm, hm_ap = {}, {}, {}, {}
                oh = p2st.tile([KH, KH * b], BF16, tag="onehot", name="oh")
                nc.sync.dma_start(out=oh[:], in_=onehot[:])
                for d in ("f", "b"):
                    wsb[d] = p2w.tile([128, KH, MG, 128], BF16, tag=f"whh_{d}", name=f"whh_{d}")
                    nc.sync.dma_start(out=wsb[d][:], in_=whh[d][:])
                    h0t = p2st.tile([128, KH, b], F32, tag=f"h0_{d}", name=f"h0_{d}")
                    nc.sync.dma_start(out=h0t[:], in_=h0[d][:])
                    hbf[d] = p2st.tile([128, KH, b], BF16, tag=f"hbf_{d}", name=f"hbf_{d}")
                    nc.scalar.copy(hbf[d][:], h0t[:])
                    hbmm[d] = p2st.tile([KH, 128], BF16, tag=f"hb_{d}", name=f"hb_{d}")
                    nc.sync.dma_start(out=hbmm[d][:], in_=hbias[d][:])
                    hm_ap[d] = h0t[:]
                ones = p2st.tile([128, KH, b], F32, tag="ones", name="ones")
                nc.vector.memset(ones[:], 1.0)

                # pin static per-engine order (the scheduler's cost model
                # mis-orders the gate chains otherwise)
                _last = {"v": None, "s": None}

                def _ord(key, bi):
                    if _last[key] is not None:
                        add_dep_helper(bi.ins, _last[key], sync=False,
                                       reason="chain order")
                    _last[key] = bi.ins
                    return bi

                SPB = 8                      # steps per block
                NBLK = T // SPB              # 32
                for blk in range(NBLK):
                    xgt, yst, s0 = {}, {}, {}
                    for d in ("f", "b"):
                        # slot range covered by this block (contiguous, len SPB)
                        s0[d] = blk * SPB if d == "f" else T - SPB - blk * SPB
                        nch, off = divmod(s0[d] * b, 512)
                        xgt[d] = p2xg.tile([128, MG, SPB * b], F32, tag=f"xgb_{d}", name=f"xgb_{d}")
                        nc.sync.dma_start(out=xgt[d][:],
                                          in_=xgc[d][nch][:, :, off:off + SPB * b])
                        yst[d] = p2y.tile([128, KH, SPB * b], F32, tag=f"yst_{d}", name=f"yst_{d}")
                    for i in range(SPB):
                        for d in ("f", "b"):
                            loc = i if d == "f" else SPB - 1 - i
                            xs = xgt[d][:, :, loc * b:(loc + 1) * b]
                            psrz = p2ps.tile([128, 2 * KH, b], F32, tag=f"psrz_{d}", name=f"psrz_{d}")
                            psn = p2ps.tile([128, KH, b], F32, tag=f"psn_{d}", name=f"psn_{d}")
                            for mc in range(2 * KH):
                                for k in range(KH):
                                    nc.tensor.matmul(
                                        psrz[:, mc, :], lhsT=wsb[d][:, k, mc, :],
                                        rhs=hbf[d][:, k, :],
                                        start=(k == 0), stop=(k == KH - 1))
                            # bias lands in PSUM first via one-hot matmul
                            nc.tensor.matmul(
                                psn[:].rearrange("p c b -> p (c b)"),
                                lhsT=hbmm[d][:], rhs=oh[:],
                                start=True, stop=False)
                            for c in range(KH):
                                for k in range(KH):
                                    nc.tensor.matmul(
                                        psn[:, c, :], lhsT=wsb[d][:, k, 2 * KH + c, :],
                                        rhs=hbf[d][:, k, :],
                                        start=False, stop=(k == KH - 1 and c == KH - 1))
                            rz = p2g.tile([128, 2 * KH, b], F32, tag=f"rz_{d}", name=f"rz_{d}")
                            _ord("v", nc.vector.tensor_add(rz[:], xs[:, 0:2 * KH, :], psrz[:]))
                            _ord("s", nc.scalar.activation(rz[:], rz[:], Sig))
                            nb = p2g.tile([128, KH, b], F32, tag=f"nb_{d}", name=f"nb_{d}")
                            _ord("v", nc.vector.scalar_tensor_tensor(
                                out=nb[:], in0=psn[:], scalar=0.0, in1=rz[:, 0:KH, :],
                                op0=mybir.AluOpType.add, op1=mybir.AluOpType.mult))
                            _ord("v", nc.vector.tensor_add(nb[:], nb[:], xs[:, 2 * KH:3 * KH, :]))
                            _ord("s", nc.scalar.activation(nb[:], nb[:], Tanh))
                            # u = 1-z, hz = h*z: off the tail, run during tanh
                            u = p2g.tile([128, KH, b], F32, tag=f"u_{d}", name=f"u_{d}")
                            _ord("v", nc.vector.scalar_tensor_tensor(
                                out=u[:], in0=rz[:, KH:2 * KH, :], scalar=-1.0, in1=ones[:],
                                op0=mybir.AluOpType.mult, op1=mybir.AluOpType.add))
                            hz = p2g.tile([128, KH, b], F32, tag=f"hz_{d}", name=f"hz_{d}")
                            _ord("v", nc.vector.tensor_mul(hz[:], hm_ap[d], rz[:, KH:2 * KH, :]))
                            _ord("v", nc.vector.tensor_mul(nb[:], nb[:], u[:]))
                            # critical: bf16 h for next step's matmuls
                            _ord("v", nc.vector.tensor_add(hbf[d][:], nb[:], hz[:]))
                            hnew = yst[d][:, :, loc * b:(loc + 1) * b]
                            nc.gpsimd.tensor_add(hnew, nb[:], hz[:])
                            hm_ap[d] = hnew
                    for d in ("f", "b"):
                        nc.sync.dma_start(
                            out=ybuf[d][:, :, s0[d] * b:(s0[d] + SPB) * b], in_=yst[d][:])
                for d in ("f", "b"):
                    nc.sync.dma_start(out=o_state[d][:], in_=hm_ap[d])

            # ---------------- Phase 3: attention pooling -------------------
            with tc.tile_pool(name="p3w", bufs=1) as p3w, \
                 tc.tile_pool(name="p3y", bufs=2) as p3y, \
                 tc.tile_pool(name="p3s", bufs=2) as p3s, \
                 tc.tile_pool(name="p3a", bufs=2) as p3a, \
                 tc.tile_pool(name="p3ps", bufs=2, space="PSUM") as p3ps:
                wi = p3w.tile([128, MJ, MJ, 128], F32R)
                nc.sync.dma_start(out=wi[:], in_=winter[:])
                bi = p3w.tile([128, MJ], F32)
                nc.sync.dma_start(out=bi[:], in_=bias_inter[:])
                wp = p3w.tile([128, MJ], F32R)
                nc.sync.dma_start(out=wp[:], in_=wproj[:])
                wf = p3w.tile([128, MJ, C], F32R)
                nc.sync.dma_start(out=wf[:], in_=wfinalT[:])
                bf_sb = p3w.tile([C, 1], F32)
                nc.sync.dma_start(out=bf_sb[:], in_=bfinal[:])
                acc = p3w.tile([C, b], F32)
                nc.vector.memset(acc[:], 0.0)

                for n in range(NT):
                    sl = slice(n * 512, (n + 1) * 512)
                    yc = {}
                    for d in ("f", "b"):
                        yc[d] = p3y.tile([128, KH, 512], F32R, tag=f"y_{d}", name=f"yc_{d}")
                        nc.sync.dma_start(out=yc[d][:], in_=ybuf[d][:, :, sl].bitcast(F32R))

                    def rhs_chunk(k):
                        d = "f" if k < KH else "b"
                        return yc[d][:, k % KH, :]

                    sq = p3s.tile([128, MJ, 512], F32R, tag="sq")
                    for m in range(MJ):
                        psq = p3ps.tile([128, 512], F32, tag="psq", bufs=3)
                        for k in range(MJ):
                            nc.tensor.matmul(psq[:], lhsT=wi[:, k, m, :],
                                             rhs=rhs_chunk(k),
                                             start=(k == 0), stop=(k == MJ - 1))
                        nc.scalar.activation(sq[:, m, :], psq[:], Tanh,
                                             bias=bi[:, m:m + 1])
                    psa = p3ps.tile([1, 512], F32, tag="psa")
                    for k in range(MJ):
                        nc.tensor.matmul(psa[:], lhsT=wp[:, k:k + 1],
                                         rhs=sq[:, k, :],
                                         start=(k == 0), stop=(k == MJ - 1))
                    psp = p3ps.tile([C, 512], F32, tag="psp")
                    for k in range(MJ):
                        nc.tensor.matmul(psp[:], lhsT=wf[:, k, :],
                                         rhs=rhs_chunk(k),
                                         start=(k == 0), stop=(k == MJ - 1))
                    asb = p3a.tile([1, 512], F32, tag="asb")
                    nc.vector.tensor_copy(asb[:], psa[:])
                    nc.sync.dma_start(out=o_attn[:, sl], in_=asb[:])
                    nc.sync.dma_start(out=attn_d[:, sl], in_=asb[:])
                    abc = p3a.tile([C, 512], F32, tag="abc")
                    asl = attn_d[0:1, sl]
                    bc_ap = bass.AP(tensor=asl.tensor, offset=asl.offset,
                                    ap=[[0, C]] + list(asl.ap)[1:])
                    nc.gpsimd.dma_start(out=abc[:], in_=bc_ap)
                    pw = p3a.tile([C, 512], F32, tag="pw")
                    nc.vector.tensor_mul(pw[:], psp[:], abc[:])
                    part = p3a.tile([C, b], F32, tag="part")
                    nc.vector.tensor_reduce(
                        part[:], pw[:].rearrange("p (t bb) -> p bb t", bb=b),
                        axis=mybir.AxisListType.X, op=mybir.AluOpType.add)
                    nc.vector.tensor_add(acc[:], acc[:], part[:])

                ofin = p3a.tile([C, b], F32, tag="ofin")
                nc.vector.tensor_scalar_add(ofin[:], acc[:], bf_sb[:, 0:1])
                nc.sync.dma_start(out=o_final[:], in_=ofin[:])

    nc.compile()
    return nc


# ----------------------------------------------------------------- entry point

def kernel(**inputs):
    if "nc" not in _CACHE:
        _CACHE["nc"] = _build()
    nc = _CACHE["nc"]

    shared = _prep_shared(inputs)
    in_maps = []
    for ci in range(NCORES):
        m = dict(shared)
        m.update(_prep_core(inputs, ci))
        in_maps.append(m)

    res = bass_utils.run_bass_kernel_spmd(
        nc, in_maps, core_ids=list(range(NCORES)), trace=TRACE)
    _CACHE["last_result"] = res

    finals, states_f, states_b, attns = [], [], [], []
    for ci in range(NCORES):
        out = res.results[ci]
        finals.append(np.asarray(out["o_final"]).T)                     # [b, C]
        states_f.append(np.asarray(out["o_state_f"]).transpose(2, 1, 0).reshape(b, H))
        states_b.append(np.asarray(out["o_state_b"]).transpose(2, 1, 0).reshape(b, H))
        attns.append(np.asarray(out["o_attn"]).reshape(T, b).T)         # [b, T]
    final_map = np.concatenate(finals, 0).astype(np.float32)
    state_out = np.stack([np.concatenate(states_f, 0),
                          np.concatenate(states_b, 0)], 0).astype(np.float32)
    attn_T = np.concatenate(attns, 0).astype(np.float32)
    return final_map, state_out, attn_T


# revision 27
# speedup vs baseline: 1.0002x; 1.0002x over previous
"""Trainium2 Bass kernel for nn_AttentionInterRNN (bidirectional GRU + attention pooling).

Strategy: data-parallel over batch B=64 across 8 NeuronCores (b=8 per core).
Per core, fully local (no collectives):
  Phase 1: xg = x @ w_ih.T + bias, computed gate-major (features on partitions)
           as fp32r matmuls, staged to DRAM.
  Phase 2: fwd+bwd GRU recurrences, interleaved per step. Recurrent matmul is
           gate-major with bf16 weights/h (fp32 PSUM accumulate, fp32 master h).
  Phase 3: attention pooling: squish = tanh(out @ W_inter + b), attn = squish @ proj,
           final = sum_t attn*(out @ W_final.T) + b_final. All fp32r matmuls.

All layout transposes are done host-side in numpy. Feature-major convention:
feature f lives at [partition p = f % 128, chunk c = f // 128].
"""
import os
import numpy as np
import ml_dtypes

import concourse.bass as bass
import concourse.tile as tile
from concourse.tile import add_dep_helper
from concourse import bacc, mybir
from concourse import bass_utils

T, B, I, H, C = 256, 64, 1024, 512, 32
NCORES = 8
b = B // NCORES          # 8 batch elements per core
TB = T * b               # 2048 flattened (t, b) per core
KI = I // 128            # 8  input chunks
KH = H // 128            # 4  hidden chunks
MG = 3 * H // 128        # 12 gate chunks
MJ = 2 * H // 128        # 8  chunks of 2H
NT = TB // 512           # 4  (t,b) chunks of 512

F32 = mybir.dt.float32
F32R = mybir.dt.float32r
BF16 = mybir.dt.bfloat16
NPBF16 = ml_dtypes.bfloat16

TRACE = os.environ.get("GRU_KERNEL_TRACE", "0") == "1"

_CACHE = {}


# ----------------------------------------------------------------- host prep

def _prep_shared(inp):
    """Weight tensors, identical on every core."""
    m = {}
    for d, suf in (("f", "_f"), ("b", "_b")):
        w_ih = np.asarray(inp["w_ih" + suf], np.float32)   # [3H, I]
        w_hh = np.asarray(inp["w_hh" + suf], np.float32)   # [3H, H]
        b_ih = np.asarray(inp["b_ih" + suf], np.float32)
        b_hh = np.asarray(inp["b_hh" + suf], np.float32)
        m["wih_" + d] = np.ascontiguousarray(
            w_ih.T.reshape(KI, 128, MG, 128).transpose(1, 0, 2, 3))
        m["whh_" + d] = np.ascontiguousarray(
            w_hh.T.reshape(KH, 128, MG, 128).transpose(1, 0, 2, 3)).astype(NPBF16)
        xb = b_ih.copy()
        xb[:2 * H] += b_hh[:2 * H]
        m["xbias_" + d] = np.ascontiguousarray(xb.reshape(MG, 128).T)
        m["hbias_" + d] = np.ascontiguousarray(
            b_hh[2 * H:].reshape(KH, 128)).astype(NPBF16)   # [4, 128] lhsT for bias MM
    m["onehot"] = np.kron(np.eye(KH, dtype=np.float32),
                          np.ones((1, b), np.float32)).astype(NPBF16)  # [4, 4*b]
    W_inter = np.asarray(inp["W_inter"], np.float32)
    m["winter"] = np.ascontiguousarray(
        W_inter.reshape(MJ, 128, MJ, 128).transpose(1, 0, 2, 3))
    m["bias_inter"] = np.ascontiguousarray(
        np.asarray(inp["bias_inter"], np.float32)[:, 0].reshape(MJ, 128).T)
    m["wproj"] = np.ascontiguousarray(
        np.asarray(inp["weight_proj"], np.float32)[:, 0].reshape(MJ, 128).T)
    m["wfinalT"] = np.ascontiguousarray(
        np.asarray(inp["W_final"], np.float32).T.reshape(MJ, 128, C).transpose(1, 0, 2))
    m["bfinal"] = np.asarray(inp["b_final"], np.float32).reshape(C, 1).copy()
    return m


def _prep_core(inp, ci):
    """Per-core input shards."""
    m = {}
    x = np.asarray(inp["intra_attention_vectors"], np.float32)[:, ci * b:(ci + 1) * b, :]
    m["xT"] = np.ascontiguousarray(
        x.transpose(2, 0, 1).reshape(KI, 128, TB).transpose(1, 0, 2))
    st = np.asarray(inp["state_inter"], np.float32)[:, ci * b:(ci + 1) * b, :]
    for di, d in enumerate(("f", "b")):
        m["h0_" + d] = np.ascontiguousarray(
            st[di].T.reshape(KH, 128, b).transpose(1, 0, 2))
    return m


# ----------------------------------------------------------------- kernel build

def _build():
    nc = bacc.Bacc("TRN2", target_bir_lowering=False, debug=False)

    # inputs
    xT = nc.dram_tensor("xT", (128, KI, TB), F32R, kind="ExternalInput").ap()
    wih = {d: nc.dram_tensor(f"wih_{d}", (128, KI, MG, 128), F32R, kind="ExternalInput").ap()
           for d in ("f", "b")}
    whh = {d: nc.dram_tensor(f"whh_{d}", (128, KH, MG, 128), BF16, kind="ExternalInput").ap()
           for d in ("f", "b")}
    h0 = {d: nc.dram_tensor(f"h0_{d}", (128, KH, b), F32, kind="ExternalInput").ap()
          for d in ("f", "b")}
    xbias = {d: nc.dram_tensor(f"xbias_{d}", (128, MG), F32, kind="ExternalInput").ap()
             for d in ("f", "b")}
    hbias = {d: nc.dram_tensor(f"hbias_{d}", (KH, 128), BF16, kind="ExternalInput").ap()
             for d in ("f", "b")}
    onehot = nc.dram_tensor("onehot", (KH, KH * b), BF16, kind="ExternalInput").ap()
    winter = nc.dram_tensor("winter", (128, MJ, MJ, 128), F32R, kind="ExternalInput").ap()
    bias_inter = nc.dram_tensor("bias_inter", (128, MJ), F32, kind="ExternalInput").ap()
    wproj = nc.dram_tensor("wproj", (128, MJ), F32R, kind="ExternalInput").ap()
    wfinalT = nc.dram_tensor("wfinalT", (128, MJ, C), F32R, kind="ExternalInput").ap()
    bfinal = nc.dram_tensor("bfinal", (C, 1), F32, kind="ExternalInput").ap()

    # outputs
    o_final = nc.dram_tensor("o_final", (C, b), F32, kind="ExternalOutput").ap()
    o_state = {d: nc.dram_tensor(f"o_state_{d}", (128, KH, b), F32, kind="ExternalOutput").ap()
               for d in ("f", "b")}
    o_attn = nc.dram_tensor("o_attn", (1, TB), F32, kind="ExternalOutput").ap()

    Ident = mybir.ActivationFunctionType.Identity
    Sig = mybir.ActivationFunctionType.Sigmoid
    Tanh = mybir.ActivationFunctionType.Tanh

    with tile.TileContext(nc) as tc:
        with tc.tile_pool(name="dram", bufs=1, space="DRAM") as dpool:
            xgc = {d: [dpool.tile([128, MG, 512], F32, name=f"xg_{d}_{n}")
                       for n in range(NT)] for d in ("f", "b")}
            ybuf = {d: dpool.tile([128, KH, TB], F32, name=f"ybuf_{d}") for d in ("f", "b")}
            attn_d = dpool.tile([1, TB], F32, name="attn_d")

            # ---------------- Phase 1: xg = w_ihT.T @ xT (+ bias) ----------
            with tc.tile_pool(name="p1w", bufs=1) as p1w, \
                 tc.tile_pool(name="p1x", bufs=1) as p1x, \
                 tc.tile_pool(name="p1s", bufs=6) as p1s, \
                 tc.tile_pool(name="p1ps", bufs=8, space="PSUM") as p1ps:
                xt = p1x.tile([128, KI, TB], F32R, name="xt")
                nc.sync.dma_start(out=xt[:], in_=xT[:])
                wt, xb = {}, {}
                for d in ("f", "b"):
                    wt[d] = p1w.tile([128, KI, MG, 128], F32R, tag=f"wih_{d}", name=f"wih_{d}")
                    nc.sync.dma_start(out=wt[d][:], in_=wih[d][:])
                    xb[d] = p1s.tile([128, MG], F32, tag=f"xbias_{d}", name=f"xb_{d}")
                    nc.sync.dma_start(out=xb[d][:], in_=xbias[d][:])
                for d, n in [("f", 0), ("b", 3), ("f", 1), ("b", 2),
                             ("f", 2), ("b", 1), ("f", 3), ("b", 0)]:
                    for m in range(MG):
                        ps1 = p1ps.tile([128, 512], F32, tag="ps", name=f"ps_{d}_{n}_{m}")
                        for k in range(KI):
                            nc.tensor.matmul(
                                ps1[:], lhsT=wt[d][:, k, m, :],
                                rhs=xt[:, k, n * 512:(n + 1) * 512],
                                start=(k == 0), stop=(k == KI - 1))
                        stg = p1s.tile([128, 512], F32, tag="stage", name=f"stg_{d}_{n}_{m}")
                        nc.scalar.activation(stg[:], ps1[:], Ident, bias=xb[d][:, m:m + 1])
                        nc.sync.dma_start(out=xgc[d][n][:, m, :], in_=stg[:])

            # ---------------- Phase 2: GRU recurrences (fwd+bwd interleaved)
            with tc.tile_pool(name="p2w", bufs=1) as p2w, \
                 tc.tile_pool(name="p2st", bufs=1) as p2st, \
                 tc.tile_pool(name="p2xg", bufs=3) as p2xg, \
                 tc.tile_pool(name="p2y", bufs=2) as p2y, \
                 tc.tile_pool(name="p2g", bufs=2) as p2g, \
                 tc.tile_pool(name="p2ps", bufs=2, space="PSUM") as p2ps:
                wsb, hbf, hbmm, hm_ap = {}, {}, {}, {}
                oh = p2st.tile([KH, KH * b], BF16, tag="onehot", name="oh")
                nc.sync.dma_start(out=oh[:], in_=onehot[:])
                for d in ("f", "b"):
                    wsb[d] = p2w.tile([128, KH, MG, 128], BF16, tag=f"whh_{d}", name=f"whh_{d}")
                    nc.sync.dma_start(out=wsb[d][:], in_=whh[d][:])
                    h0t = p2st.tile([128, KH, b], F32, tag=f"h0_{d}", name=f"h0_{d}")
                    nc.sync.dma_start(out=h0t[:], in_=h0[d][:])
                    hbf[d] = p2st.tile([128, KH, b], BF16, tag=f"hbf_{d}", name=f"hbf_{d}")
                    nc.scalar.copy(hbf[d][:], h0t[:])
                    hbmm[d] = p2st.tile([KH, 128], BF16, tag=f"hb_{d}", name=f"hb_{d}")
                    nc.sync.dma_start(out=hbmm[d][:], in_=hbias[d][:])
                    hm_ap[d] = h0t[:]
                ones = p2st.tile([128, KH, b], F32, tag="ones", name="ones")
                nc.vector.memset(ones[:], 1.0)

                # pin static per-engine order (the scheduler's cost model
                # mis-orders the gate chains otherwise)
                _last = {"v": None, "s": None}

                def _ord(key, bi):
                    if _last[key] is not None:
                        add_dep_helper(bi.ins, _last[key], sync=False,
                                       reason="chain order")
                    _last[key] = bi.ins
                    return bi

                SPB = 8                      # steps per block
                NBLK = T // SPB              # 32
                for blk in range(NBLK):
                    xgt, yst, s0 = {}, {}, {}
                    for d in ("f", "b"):
                        # slot range covered by this block (contiguous, len SPB)
                        s0[d] = blk * SPB if d == "f" else T - SPB - blk * SPB
                        nch, off = divmod(s0[d] * b, 512)
                        xgt[d] = p2xg.tile([128, MG, SPB * b], F32, tag=f"xgb_{d}", name=f"xgb_{d}")
                        nc.sync.dma_start(out=xgt[d][:],
                                          in_=xgc[d][nch][:, :, off:off + SPB * b])
                        yst[d] = p2y.tile([128, KH, SPB * b], F32, tag=f"yst_{d}", name=f"yst_{d}")
                    for i in range(SPB):
                        for d in ("f", "b"):
                            loc = i if d == "f" else SPB - 1 - i
                            xs = xgt[d][:, :, loc * b:(loc + 1) * b]
                            psrz = p2ps.tile([128, 2 * KH, b], F32, tag=f"psrz_{d}", name=f"psrz_{d}")
                            psn = p2ps.tile([128, KH, b], F32, tag=f"psn_{d}", name=f"psn_{d}")
                            for mc in range(2 * KH):
                                for k in range(KH):
                                    nc.tensor.matmul(
                                        psrz[:, mc, :], lhsT=wsb[d][:, k, mc, :],
                                        rhs=hbf[d][:, k, :],
                                        start=(k == 0), stop=(k == KH - 1))
                            # bias lands in PSUM first via one-hot matmul
                            nc.tensor.matmul(
                                psn[:].rearrange("p c b -> p (c b)"),
                                lhsT=hbmm[d][:], rhs=oh[:],
                                start=True, stop=False)
                            for c in range(KH):
                                for k in range(KH):
                                    nc.tensor.matmul(
                                        psn[:, c, :], lhsT=wsb[d][:, k, 2 * KH + c, :],
                                        rhs=hbf[d][:, k, :],
                                        start=False, stop=(k == KH - 1 and c == KH - 1))
                            rz = p2g.tile([128, 2 * KH, b], F32, tag=f"rz_{d}", name=f"rz_{d}")
                            _ord("v", nc.vector.tensor_add(rz[:], xs[:, 0:2 * KH, :], psrz[:]))
                            _ord("s", nc.scalar.activation(rz[:], rz[:], Sig))
                            nb = p2g.tile([128, KH, b], F32, tag=f"nb_{d}", name=f"nb_{d}")
                            _ord("v", nc.vector.scalar_tensor_tensor(
                                out=nb[:], in0=psn[:], scalar=0.0, in1=rz[:, 0:KH, :],
                                op0=mybir.AluOpType.add, op1=mybir.AluOpType.mult))
                            _ord("v", nc.vector.tensor_add(nb[:], nb[:], xs[:, 2 * KH:3 * KH, :]))
                            _ord("s", nc.scalar.activation(nb[:], nb[:], Tanh))
                            # u = 1-z, hz = h*z: off the tail, run during tanh
                            u = p2g.tile([128, KH, b], F32, tag=f"u_{d}", name=f"u_{d}")
                            _ord("v", nc.vector.scalar_tensor_tensor(
                                out=u[:], in0=rz[:, KH:2 * KH, :], scalar=-1.0, in1=ones[:],
                                op0=mybir.AluOpType.mult, op1=mybir.AluOpType.add))
                            hz = p2g.tile([128, KH, b], F32, tag=f"hz_{d}", name=f"hz_{d}")
                            _ord("v", nc.vector.tensor_mul(hz[:], hm_ap[d], rz[:, KH:2 * KH, :]))
                            _ord("v", nc.vector.tensor_mul(nb[:], nb[:], u[:]))
                            # critical: bf16 h for next step's matmuls
                            _ord("v", nc.vector.tensor_add(hbf[d][:], nb[:], hz[:]))
                            hnew = yst[d][:, :, loc * b:(loc + 1) * b]
                            nc.gpsimd.tensor_add(hnew, nb[:], hz[:])
                            hm_ap[d] = hnew
                    for d in ("f", "b"):
                        nc.sync.dma_start(
                            out=ybuf[d][:, :, s0[d] * b:(s0[d] + SPB) * b], in_=yst[d][:])
                for d in ("f", "b"):
                    nc.sync.dma_start(out=o_state[d][:], in_=hm_ap[d])

            # ---------------- Phase 3: attention pooling -------------------
            with tc.tile_pool(name="p3w", bufs=1) as p3w, \
                 tc.tile_pool(name="p3y", bufs=2) as p3y, \
                 tc.tile_pool(name="p3s", bufs=2) as p3s, \
                 tc.tile_pool(name="p3a", bufs=2) as p3a, \
                 tc.tile_pool(name="p3ps", bufs=2, space="PSUM") as p3ps:
                wi = p3w.tile([128, MJ, MJ, 128], F32R)
                nc.sync.dma_start(out=wi[:], in_=winter[:])
                bi = p3w.tile([128, MJ], F32)
                nc.sync.dma_start(out=bi[:], in_=bias_inter[:])
                wp = p3w.tile([128, MJ], F32R)
                nc.sync.dma_start(out=wp[:], in_=wproj[:])
                wf = p3w.tile([128, MJ, C], F32R)
                nc.sync.dma_start(out=wf[:], in_=wfinalT[:])
                bf_sb = p3w.tile([C, 1], F32)
                nc.sync.dma_start(out=bf_sb[:], in_=bfinal[:])
                acc = p3w.tile([C, b], F32)
                nc.vector.memset(acc[:], 0.0)

                for n in range(NT):
                    sl = slice(n * 512, (n + 1) * 512)
                    yc = {}
                    for d in ("f", "b"):
                        yc[d] = p3y.tile([128, KH, 512], F32R, tag=f"y_{d}", name=f"yc_{d}")
                        nc.sync.dma_start(out=yc[d][:], in_=ybuf[d][:, :, sl].bitcast(F32R))

                    def rhs_chunk(k):
                        d = "f" if k < KH else "b"
                        return yc[d][:, k % KH, :]

                    sq = p3s.tile([128, MJ, 512], F32R, tag="sq")
                    for m in range(MJ):
                        psq = p3ps.tile([128, 512], F32, tag="psq")
                        for k in range(MJ):
                            nc.tensor.matmul(psq[:], lhsT=wi[:, k, m, :],
                                             rhs=rhs_chunk(k),
                                             start=(k == 0), stop=(k == MJ - 1))
                        nc.scalar.activation(sq[:, m, :], psq[:], Tanh,
                                             bias=bi[:, m:m + 1])
                    psa = p3ps.tile([1, 512], F32, tag="psa")
                    for k in range(MJ):
                        nc.tensor.matmul(psa[:], lhsT=wp[:, k:k + 1],
                                         rhs=sq[:, k, :],
                                         start=(k == 0), stop=(k == MJ - 1))
                    psp = p3ps.tile([C, 512], F32, tag="psp")
                    for k in range(MJ):
                        nc.tensor.matmul(psp[:], lhsT=wf[:, k, :],
                                         rhs=rhs_chunk(k),
                                         start=(k == 0), stop=(k == MJ - 1))
                    asb = p3a.tile([1, 512], F32, tag="asb")
                    nc.vector.tensor_copy(asb[:], psa[:])
                    nc.sync.dma_start(out=o_attn[:, sl], in_=asb[:])
                    nc.sync.dma_start(out=attn_d[:, sl], in_=asb[:])
                    abc = p3a.tile([C, 512], F32, tag="abc")
                    asl = attn_d[0:1, sl]
                    bc_ap = bass.AP(tensor=asl.tensor, offset=asl.offset,
                                    ap=[[0, C]] + list(asl.ap)[1:])
                    nc.gpsimd.dma_start(out=abc[:], in_=bc_ap)
                    pw = p3a.tile([C, 512], F32, tag="pw")
                    nc.vector.tensor_mul(pw[:], psp[:], abc[:])
                    part = p3a.tile([C, b], F32, tag="part")
                    nc.vector.tensor_reduce(
                        part[:], pw[:].rearrange("p (t bb) -> p bb t", bb=b),
                        axis=mybir.AxisListType.X, op=mybir.AluOpType.add)
                    nc.vector.tensor_add(acc[:], acc[:], part[:])

                ofin = p3a.tile([C, b], F32, tag="ofin")
                nc.vector.tensor_scalar_add(ofin[:], acc[:], bf_sb[:, 0:1])
                nc.sync.dma_start(out=o_final[:], in_=ofin[:])

    nc.compile()
    return nc


# ----------------------------------------------------------------- entry point

def kernel(**inputs):
    if "nc" not in _CACHE:
        _CACHE["nc"] = _build()
    nc = _CACHE["nc"]

    shared = _prep_shared(inputs)
    in_maps = []
    for ci in range(NCORES):
        m = dict(shared)
        m.update(_prep_core(inputs, ci))
        in_maps.append(m)

    res = bass_utils.run_bass_kernel_spmd(
        nc, in_maps, core_ids=list(range(NCORES)), trace=TRACE)
    _CACHE["last_result"] = res

    finals, states_f, states_b, attns = [], [], [], []
    for ci in range(NCORES):
        out = res.results[ci]
        finals.append(np.asarray(out["o_final"]).T)                     # [b, C]
        states_f.append(np.asarray(out["o_state_f"]).transpose(2, 1, 0).reshape(b, H))
        states_b.append(np.asarray(out["o_state_b"]).transpose(2, 1, 0).reshape(b, H))
        attns.append(np.asarray(out["o_attn"]).reshape(T, b).T)         # [b, T]
    final_map = np.concatenate(finals, 0).astype(np.float32)
    state_out = np.stack([np.concatenate(states_f, 0),
                          np.concatenate(states_b, 0)], 0).astype(np.float32)
    attn_T = np.concatenate(attns, 0).astype(np.float32)
    return final_map, state_out, attn_T


# revision 28
# speedup vs baseline: 1.0114x; 1.0111x over previous
"""Trainium2 Bass kernel for nn_AttentionInterRNN (bidirectional GRU + attention pooling).

Strategy: data-parallel over batch B=64 across 8 NeuronCores (b=8 per core).
Per core, fully local (no collectives):
  Phase 1: xg = x @ w_ih.T + bias, computed gate-major (features on partitions)
           as fp32r matmuls, staged to DRAM.
  Phase 2: fwd+bwd GRU recurrences, interleaved per step. Recurrent matmul is
           gate-major with bf16 weights/h (fp32 PSUM accumulate, fp32 master h).
  Phase 3: attention pooling: squish = tanh(out @ W_inter + b), attn = squish @ proj,
           final = sum_t attn*(out @ W_final.T) + b_final. All fp32r matmuls.

All layout transposes are done host-side in numpy. Feature-major convention:
feature f lives at [partition p = f % 128, chunk c = f // 128].
"""
import os
import numpy as np
import ml_dtypes

import concourse.bass as bass
import concourse.tile as tile
from concourse.tile import add_dep_helper
from concourse import bacc, mybir
from concourse import bass_utils

T, B, I, H, C = 256, 64, 1024, 512, 32
NCORES = 8
b = B // NCORES          # 8 batch elements per core
TB = T * b               # 2048 flattened (t, b) per core
KI = I // 128            # 8  input chunks
KH = H // 128            # 4  hidden chunks
MG = 3 * H // 128        # 12 gate chunks
MJ = 2 * H // 128        # 8  chunks of 2H
NT = TB // 512           # 4  (t,b) chunks of 512

F32 = mybir.dt.float32
F32R = mybir.dt.float32r
BF16 = mybir.dt.bfloat16
NPBF16 = ml_dtypes.bfloat16

TRACE = os.environ.get("GRU_KERNEL_TRACE", "0") == "1"

_CACHE = {}


# ----------------------------------------------------------------- host prep

def _prep_shared(inp):
    """Weight tensors, identical on every core."""
    m = {}
    for d, suf in (("f", "_f"), ("b", "_b")):
        w_ih = np.asarray(inp["w_ih" + suf], np.float32)   # [3H, I]
        w_hh = np.asarray(inp["w_hh" + suf], np.float32)   # [3H, H]
        b_ih = np.asarray(inp["b_ih" + suf], np.float32)
        b_hh = np.asarray(inp["b_hh" + suf], np.float32)
        m["wih_" + d] = np.ascontiguousarray(
            w_ih.T.reshape(KI, 128, MG, 128).transpose(1, 0, 2, 3))
        m["whh_" + d] = np.ascontiguousarray(
            w_hh.T.reshape(KH, 128, MG, 128).transpose(1, 0, 2, 3)).astype(NPBF16)
        xb = b_ih.copy()
        xb[:2 * H] += b_hh[:2 * H]
        m["xbias_" + d] = np.ascontiguousarray(xb.reshape(MG, 128).T)
        m["hbias_" + d] = np.ascontiguousarray(
            b_hh[2 * H:].reshape(KH, 128)).astype(NPBF16)   # [4, 128] lhsT for bias MM
    m["onehot"] = np.kron(np.eye(KH, dtype=np.float32),
                          np.ones((1, b), np.float32)).astype(NPBF16)  # [4, 4*b]
    W_inter = np.asarray(inp["W_inter"], np.float32)
    m["winter"] = np.ascontiguousarray(
        W_inter.reshape(MJ, 128, MJ, 128).transpose(1, 0, 2, 3))
    m["bias_inter"] = np.ascontiguousarray(
        np.asarray(inp["bias_inter"], np.float32)[:, 0].reshape(MJ, 128).T)
    m["wproj"] = np.ascontiguousarray(
        np.asarray(inp["weight_proj"], np.float32)[:, 0].reshape(MJ, 128).T)
    m["wfinalT"] = np.ascontiguousarray(
        np.asarray(inp["W_final"], np.float32).T.reshape(MJ, 128, C).transpose(1, 0, 2))
    m["bfinal"] = np.asarray(inp["b_final"], np.float32).reshape(C, 1).copy()
    return m


def _prep_core(inp, ci):
    """Per-core input shards."""
    m = {}
    x = np.asarray(inp["intra_attention_vectors"], np.float32)[:, ci * b:(ci + 1) * b, :]
    m["xT"] = np.ascontiguousarray(
        x.transpose(2, 0, 1).reshape(KI, 128, TB).transpose(1, 0, 2))
    st = np.asarray(inp["state_inter"], np.float32)[:, ci * b:(ci + 1) * b, :]
    for di, d in enumerate(("f", "b")):
        m["h0_" + d] = np.ascontiguousarray(
            st[di].T.reshape(KH, 128, b).transpose(1, 0, 2))
    return m


# ----------------------------------------------------------------- kernel build

def _build():
    nc = bacc.Bacc("TRN2", target_bir_lowering=False, debug=False)

    # inputs
    xT = nc.dram_tensor("xT", (128, KI, TB), F32R, kind="ExternalInput").ap()
    wih = {d: nc.dram_tensor(f"wih_{d}", (128, KI, MG, 128), F32R, kind="ExternalInput").ap()
           for d in ("f", "b")}
    whh = {d: nc.dram_tensor(f"whh_{d}", (128, KH, MG, 128), BF16, kind="ExternalInput").ap()
           for d in ("f", "b")}
    h0 = {d: nc.dram_tensor(f"h0_{d}", (128, KH, b), F32, kind="ExternalInput").ap()
          for d in ("f", "b")}
    xbias = {d: nc.dram_tensor(f"xbias_{d}", (128, MG), F32, kind="ExternalInput").ap()
             for d in ("f", "b")}
    hbias = {d: nc.dram_tensor(f"hbias_{d}", (KH, 128), BF16, kind="ExternalInput").ap()
             for d in ("f", "b")}
    onehot = nc.dram_tensor("onehot", (KH, KH * b), BF16, kind="ExternalInput").ap()
    winter = nc.dram_tensor("winter", (128, MJ, MJ, 128), F32R, kind="ExternalInput").ap()
    bias_inter = nc.dram_tensor("bias_inter", (128, MJ), F32, kind="ExternalInput").ap()
    wproj = nc.dram_tensor("wproj", (128, MJ), F32R, kind="ExternalInput").ap()
    wfinalT = nc.dram_tensor("wfinalT", (128, MJ, C), F32R, kind="ExternalInput").ap()
    bfinal = nc.dram_tensor("bfinal", (C, 1), F32, kind="ExternalInput").ap()

    # outputs
    o_final = nc.dram_tensor("o_final", (C, b), F32, kind="ExternalOutput").ap()
    o_state = {d: nc.dram_tensor(f"o_state_{d}", (128, KH, b), F32, kind="ExternalOutput").ap()
               for d in ("f", "b")}
    o_attn = nc.dram_tensor("o_attn", (1, TB), F32, kind="ExternalOutput").ap()

    Ident = mybir.ActivationFunctionType.Identity
    Sig = mybir.ActivationFunctionType.Sigmoid
    Tanh = mybir.ActivationFunctionType.Tanh

    with tile.TileContext(nc) as tc:
        with tc.tile_pool(name="dram", bufs=1, space="DRAM") as dpool:
            xgc = {d: [dpool.tile([128, MG, 512], F32, name=f"xg_{d}_{n}")
                       for n in range(NT)] for d in ("f", "b")}
            ybuf = {d: dpool.tile([128, KH, TB], F32, name=f"ybuf_{d}") for d in ("f", "b")}
            attn_d = dpool.tile([1, TB], F32, name="attn_d")

            # ---------------- Phase 1: xg = w_ihT.T @ xT (+ bias) ----------
            with tc.tile_pool(name="p1w", bufs=1) as p1w, \
                 tc.tile_pool(name="p1x", bufs=1) as p1x, \
                 tc.tile_pool(name="p1s", bufs=6) as p1s, \
                 tc.tile_pool(name="p1ps", bufs=8, space="PSUM") as p1ps:
                xt = p1x.tile([128, KI, TB], F32R, name="xt")
                wt, xb = {}, {}
                for d in ("f", "b"):
                    wt[d] = p1w.tile([128, KI, MG, 128], F32R, tag=f"wih_{d}", name=f"wih_{d}")
                    xb[d] = p1s.tile([128, MG], F32, tag=f"xbias_{d}", name=f"xb_{d}")
                    nc.sync.dma_start(out=xb[d][:], in_=xbias[d][:])
                # piecewise input loads, first-needed pieces first, so the
                # first matmuls start ~8us in instead of ~50us
                nc.sync.dma_start(out=xt[:, :, 0:512], in_=xT[:, :, 0:512])
                for k in range(KI):
                    nc.sync.dma_start(out=wt["f"][:, k], in_=wih["f"][:, k])
                nc.sync.dma_start(out=xt[:, :, 3 * 512:4 * 512], in_=xT[:, :, 3 * 512:4 * 512])
                for k in range(KI):
                    nc.sync.dma_start(out=wt["b"][:, k], in_=wih["b"][:, k])
                for n in (1, 2):
                    nc.sync.dma_start(out=xt[:, :, n * 512:(n + 1) * 512],
                                      in_=xT[:, :, n * 512:(n + 1) * 512])
                for d, n in [("f", 0), ("b", 3), ("f", 1), ("b", 2),
                             ("f", 2), ("b", 1), ("f", 3), ("b", 0)]:
                    for m in range(MG):
                        ps1 = p1ps.tile([128, 512], F32, tag="ps", name=f"ps_{d}_{n}_{m}")
                        for k in range(KI):
                            nc.tensor.matmul(
                                ps1[:], lhsT=wt[d][:, k, m, :],
                                rhs=xt[:, k, n * 512:(n + 1) * 512],
                                start=(k == 0), stop=(k == KI - 1))
                        stg = p1s.tile([128, 512], F32, tag="stage", name=f"stg_{d}_{n}_{m}")
                        nc.scalar.activation(stg[:], ps1[:], Ident, bias=xb[d][:, m:m + 1])
                        nc.sync.dma_start(out=xgc[d][n][:, m, :], in_=stg[:])

            # ---------------- Phase 2: GRU recurrences (fwd+bwd interleaved)
            with tc.tile_pool(name="p2w", bufs=1) as p2w, \
                 tc.tile_pool(name="p2st", bufs=1) as p2st, \
                 tc.tile_pool(name="p2xg", bufs=3) as p2xg, \
                 tc.tile_pool(name="p2y", bufs=2) as p2y, \
                 tc.tile_pool(name="p2g", bufs=2) as p2g, \
                 tc.tile_pool(name="p2ps", bufs=2, space="PSUM") as p2ps:
                wsb, hbf, hbmm, hm_ap = {}, {}, {}, {}
                oh = p2st.tile([KH, KH * b], BF16, tag="onehot", name="oh")
                nc.sync.dma_start(out=oh[:], in_=onehot[:])
                for d in ("f", "b"):
                    wsb[d] = p2w.tile([128, KH, MG, 128], BF16, tag=f"whh_{d}", name=f"whh_{d}")
                    nc.sync.dma_start(out=wsb[d][:], in_=whh[d][:])
                    h0t = p2st.tile([128, KH, b], F32, tag=f"h0_{d}", name=f"h0_{d}")
                    nc.sync.dma_start(out=h0t[:], in_=h0[d][:])
                    hbf[d] = p2st.tile([128, KH, b], BF16, tag=f"hbf_{d}", name=f"hbf_{d}")
                    nc.scalar.copy(hbf[d][:], h0t[:])
                    hbmm[d] = p2st.tile([KH, 128], BF16, tag=f"hb_{d}", name=f"hb_{d}")
                    nc.sync.dma_start(out=hbmm[d][:], in_=hbias[d][:])
                    hm_ap[d] = h0t[:]
                ones = p2st.tile([128, KH, b], F32, tag="ones", name="ones")
                nc.vector.memset(ones[:], 1.0)

                # pin static per-engine order (the scheduler's cost model
                # mis-orders the gate chains otherwise)
                _last = {"v": None, "s": None}

                def _ord(key, bi):
                    if _last[key] is not None:
                        add_dep_helper(bi.ins, _last[key], sync=False,
                                       reason="chain order")
                    _last[key] = bi.ins
                    return bi

                SPB = 8                      # steps per block
                NBLK = T // SPB              # 32
                for blk in range(NBLK):
                    xgt, yst, s0 = {}, {}, {}
                    for d in ("f", "b"):
                        # slot range covered by this block (contiguous, len SPB)
                        s0[d] = blk * SPB if d == "f" else T - SPB - blk * SPB
                        nch, off = divmod(s0[d] * b, 512)
                        xgt[d] = p2xg.tile([128, MG, SPB * b], F32, tag=f"xgb_{d}", name=f"xgb_{d}")
                        nc.sync.dma_start(out=xgt[d][:],
                                          in_=xgc[d][nch][:, :, off:off + SPB * b])
                        yst[d] = p2y.tile([128, KH, SPB * b], F32, tag=f"yst_{d}", name=f"yst_{d}")
                    for i in range(SPB):
                        for d in ("f", "b"):
                            loc = i if d == "f" else SPB - 1 - i
                            xs = xgt[d][:, :, loc * b:(loc + 1) * b]
                            psrz = p2ps.tile([128, 2 * KH, b], F32, tag=f"psrz_{d}", name=f"psrz_{d}")
                            psn = p2ps.tile([128, KH, b], F32, tag=f"psn_{d}", name=f"psn_{d}")
                            for mc in range(2 * KH):
                                for k in range(KH):
                                    nc.tensor.matmul(
                                        psrz[:, mc, :], lhsT=wsb[d][:, k, mc, :],
                                        rhs=hbf[d][:, k, :],
                                        start=(k == 0), stop=(k == KH - 1))
                            # bias lands in PSUM first via one-hot matmul
                            nc.tensor.matmul(
                                psn[:].rearrange("p c b -> p (c b)"),
                                lhsT=hbmm[d][:], rhs=oh[:],
                                start=True, stop=False)
                            for c in range(KH):
                                for k in range(KH):
                                    nc.tensor.matmul(
                                        psn[:, c, :], lhsT=wsb[d][:, k, 2 * KH + c, :],
                                        rhs=hbf[d][:, k, :],
                                        start=False, stop=(k == KH - 1 and c == KH - 1))
                            rz = p2g.tile([128, 2 * KH, b], F32, tag=f"rz_{d}", name=f"rz_{d}")
                            _ord("v", nc.vector.tensor_add(rz[:], xs[:, 0:2 * KH, :], psrz[:]))
                            _ord("s", nc.scalar.activation(rz[:], rz[:], Sig))
                            nb = p2g.tile([128, KH, b], F32, tag=f"nb_{d}", name=f"nb_{d}")
                            _ord("v", nc.vector.scalar_tensor_tensor(
                                out=nb[:], in0=psn[:], scalar=0.0, in1=rz[:, 0:KH, :],
                                op0=mybir.AluOpType.add, op1=mybir.AluOpType.mult))
                            _ord("v", nc.vector.tensor_add(nb[:], nb[:], xs[:, 2 * KH:3 * KH, :]))
                            _ord("s", nc.scalar.activation(nb[:], nb[:], Tanh))
                            # u = 1-z, hz = h*z: off the tail, run during tanh
                            u = p2g.tile([128, KH, b], F32, tag=f"u_{d}", name=f"u_{d}")
                            _ord("v", nc.vector.scalar_tensor_tensor(
                                out=u[:], in0=rz[:, KH:2 * KH, :], scalar=-1.0, in1=ones[:],
                                op0=mybir.AluOpType.mult, op1=mybir.AluOpType.add))
                            hz = p2g.tile([128, KH, b], F32, tag=f"hz_{d}", name=f"hz_{d}")
                            _ord("v", nc.vector.tensor_mul(hz[:], hm_ap[d], rz[:, KH:2 * KH, :]))
                            _ord("v", nc.vector.tensor_mul(nb[:], nb[:], u[:]))
                            # critical: bf16 h for next step's matmuls
                            _ord("v", nc.vector.tensor_add(hbf[d][:], nb[:], hz[:]))
                            hnew = yst[d][:, :, loc * b:(loc + 1) * b]
                            nc.gpsimd.tensor_add(hnew, nb[:], hz[:])
                            hm_ap[d] = hnew
                    for d in ("f", "b"):
                        nc.sync.dma_start(
                            out=ybuf[d][:, :, s0[d] * b:(s0[d] + SPB) * b], in_=yst[d][:])
                for d in ("f", "b"):
                    nc.sync.dma_start(out=o_state[d][:], in_=hm_ap[d])

            # ---------------- Phase 3: attention pooling -------------------
            with tc.tile_pool(name="p3w", bufs=1) as p3w, \
                 tc.tile_pool(name="p3y", bufs=2) as p3y, \
                 tc.tile_pool(name="p3s", bufs=2) as p3s, \
                 tc.tile_pool(name="p3a", bufs=2) as p3a, \
                 tc.tile_pool(name="p3ps", bufs=2, space="PSUM") as p3ps:
                wi = p3w.tile([128, MJ, MJ, 128], F32R)
                nc.sync.dma_start(out=wi[:], in_=winter[:])
                bi = p3w.tile([128, MJ], F32)
                nc.sync.dma_start(out=bi[:], in_=bias_inter[:])
                wp = p3w.tile([128, MJ], F32R)
                nc.sync.dma_start(out=wp[:], in_=wproj[:])
                wf = p3w.tile([128, MJ, C], F32R)
                nc.sync.dma_start(out=wf[:], in_=wfinalT[:])
                bf_sb = p3w.tile([C, 1], F32)
                nc.sync.dma_start(out=bf_sb[:], in_=bfinal[:])
                acc = p3w.tile([C, b], F32)
                nc.vector.memset(acc[:], 0.0)

                for n in range(NT):
                    sl = slice(n * 512, (n + 1) * 512)
                    yc = {}
                    for d in ("f", "b"):
                        yc[d] = p3y.tile([128, KH, 512], F32R, tag=f"y_{d}", name=f"yc_{d}")
                        nc.sync.dma_start(out=yc[d][:], in_=ybuf[d][:, :, sl].bitcast(F32R))

                    def rhs_chunk(k):
                        d = "f" if k < KH else "b"
                        return yc[d][:, k % KH, :]

                    sq = p3s.tile([128, MJ, 512], F32R, tag="sq")
                    for m in range(MJ):
                        psq = p3ps.tile([128, 512], F32, tag="psq")
                        for k in range(MJ):
                            nc.tensor.matmul(psq[:], lhsT=wi[:, k, m, :],
                                             rhs=rhs_chunk(k),
                                             start=(k == 0), stop=(k == MJ - 1))
                        nc.scalar.activation(sq[:, m, :], psq[:], Tanh,
                                             bias=bi[:, m:m + 1])
                    psa = p3ps.tile([1, 512], F32, tag="psa")
                    for k in range(MJ):
                        nc.tensor.matmul(psa[:], lhsT=wp[:, k:k + 1],
                                         rhs=sq[:, k, :],
                                         start=(k == 0), stop=(k == MJ - 1))
                    psp = p3ps.tile([C, 512], F32, tag="psp")
                    for k in range(MJ):
                        nc.tensor.matmul(psp[:], lhsT=wf[:, k, :],
                                         rhs=rhs_chunk(k),
                                         start=(k == 0), stop=(k == MJ - 1))
                    asb = p3a.tile([1, 512], F32, tag="asb")
                    nc.vector.tensor_copy(asb[:], psa[:])
                    nc.sync.dma_start(out=o_attn[:, sl], in_=asb[:])
                    nc.sync.dma_start(out=attn_d[:, sl], in_=asb[:])
                    abc = p3a.tile([C, 512], F32, tag="abc")
                    asl = attn_d[0:1, sl]
                    bc_ap = bass.AP(tensor=asl.tensor, offset=asl.offset,
                                    ap=[[0, C]] + list(asl.ap)[1:])
                    nc.gpsimd.dma_start(out=abc[:], in_=bc_ap)
                    pw = p3a.tile([C, 512], F32, tag="pw")
                    nc.vector.tensor_mul(pw[:], psp[:], abc[:])
                    part = p3a.tile([C, b], F32, tag="part")
                    nc.vector.tensor_reduce(
                        part[:], pw[:].rearrange("p (t bb) -> p bb t", bb=b),
                        axis=mybir.AxisListType.X, op=mybir.AluOpType.add)
                    nc.vector.tensor_add(acc[:], acc[:], part[:])

                ofin = p3a.tile([C, b], F32, tag="ofin")
                nc.vector.tensor_scalar_add(ofin[:], acc[:], bf_sb[:, 0:1])
                nc.sync.dma_start(out=o_final[:], in_=ofin[:])

    nc.compile()
    return nc


# ----------------------------------------------------------------- entry point

def kernel(**inputs):
    if "nc" not in _CACHE:
        _CACHE["nc"] = _build()
    nc = _CACHE["nc"]

    shared = _prep_shared(inputs)
    in_maps = []
    for ci in range(NCORES):
        m = dict(shared)
        m.update(_prep_core(inputs, ci))
        in_maps.append(m)

    res = bass_utils.run_bass_kernel_spmd(
        nc, in_maps, core_ids=list(range(NCORES)), trace=TRACE)
    _CACHE["last_result"] = res

    finals, states_f, states_b, attns = [], [], [], []
    for ci in range(NCORES):
        out = res.results[ci]
        finals.append(np.asarray(out["o_final"]).T)                     # [b, C]
        states_f.append(np.asarray(out["o_state_f"]).transpose(2, 1, 0).reshape(b, H))
        states_b.append(np.asarray(out["o_state_b"]).transpose(2, 1, 0).reshape(b, H))
        attns.append(np.asarray(out["o_attn"]).reshape(T, b).T)         # [b, T]
    final_map = np.concatenate(finals, 0).astype(np.float32)
    state_out = np.stack([np.concatenate(states_f, 0),
                          np.concatenate(states_b, 0)], 0).astype(np.float32)
    attn_T = np.concatenate(attns, 0).astype(np.float32)
    return final_map, state_out, attn_T


# revision 29
# speedup vs baseline: 1.0386x; 1.0269x over previous
"""Trainium2 Bass kernel for nn_AttentionInterRNN (bidirectional GRU + attention pooling).

Strategy: data-parallel over batch B=64 across 8 NeuronCores (b=8 per core).
Per core, fully local (no collectives):
  Phase 1: xg = x @ w_ih.T + bias, computed gate-major (features on partitions)
           as fp32r matmuls, staged to DRAM.
  Phase 2: fwd+bwd GRU recurrences, interleaved per step. Recurrent matmul is
           gate-major with bf16 weights/h (fp32 PSUM accumulate, fp32 master h).
  Phase 3: attention pooling: squish = tanh(out @ W_inter + b), attn = squish @ proj,
           final = sum_t attn*(out @ W_final.T) + b_final. All fp32r matmuls.

All layout transposes are done host-side in numpy. Feature-major convention:
feature f lives at [partition p = f % 128, chunk c = f // 128].
"""
import os
import numpy as np
import ml_dtypes

import concourse.bass as bass
import concourse.tile as tile
from concourse.tile import add_dep_helper
from concourse import bacc, mybir
from concourse import bass_utils

T, B, I, H, C = 256, 64, 1024, 512, 32
NCORES = 8
b = B // NCORES          # 8 batch elements per core
TB = T * b               # 2048 flattened (t, b) per core
KI = I // 128            # 8  input chunks
KH = H // 128            # 4  hidden chunks
MG = 3 * H // 128        # 12 gate chunks
MJ = 2 * H // 128        # 8  chunks of 2H
NT = TB // 512           # 4  (t,b) chunks of 512

F32 = mybir.dt.float32
F32R = mybir.dt.float32r
BF16 = mybir.dt.bfloat16
NPBF16 = ml_dtypes.bfloat16

TRACE = os.environ.get("GRU_KERNEL_TRACE", "0") == "1"

_CACHE = {}


# ----------------------------------------------------------------- host prep

def _prep_shared(inp):
    """Weight tensors, identical on every core."""
    m = {}
    for d, suf in (("f", "_f"), ("b", "_b")):
        w_ih = np.asarray(inp["w_ih" + suf], np.float32)   # [3H, I]
        w_hh = np.asarray(inp["w_hh" + suf], np.float32)   # [3H, H]
        b_ih = np.asarray(inp["b_ih" + suf], np.float32)
        b_hh = np.asarray(inp["b_hh" + suf], np.float32)
        m["wih_" + d] = np.ascontiguousarray(
            w_ih.T.reshape(KI, 128, MG, 128).transpose(1, 0, 2, 3)).astype(NPBF16)
        m["whh_" + d] = np.ascontiguousarray(
            w_hh.T.reshape(KH, 128, MG, 128).transpose(1, 0, 2, 3)).astype(NPBF16)
        xb = b_ih.copy()
        xb[:2 * H] += b_hh[:2 * H]
        m["xbias_" + d] = np.ascontiguousarray(xb.reshape(MG, 128).T)
        m["hbias_" + d] = np.ascontiguousarray(
            b_hh[2 * H:].reshape(KH, 128)).astype(NPBF16)   # [4, 128] lhsT for bias MM
    m["onehot"] = np.kron(np.eye(KH, dtype=np.float32),
                          np.ones((1, b), np.float32)).astype(NPBF16)  # [4, 4*b]
    W_inter = np.asarray(inp["W_inter"], np.float32)
    m["winter"] = np.ascontiguousarray(
        W_inter.reshape(MJ, 128, MJ, 128).transpose(1, 0, 2, 3))
    m["bias_inter"] = np.ascontiguousarray(
        np.asarray(inp["bias_inter"], np.float32)[:, 0].reshape(MJ, 128).T)
    m["wproj"] = np.ascontiguousarray(
        np.asarray(inp["weight_proj"], np.float32)[:, 0].reshape(MJ, 128).T)
    m["wfinalT"] = np.ascontiguousarray(
        np.asarray(inp["W_final"], np.float32).T.reshape(MJ, 128, C).transpose(1, 0, 2))
    m["bfinal"] = np.asarray(inp["b_final"], np.float32).reshape(C, 1).copy()
    return m


def _prep_core(inp, ci):
    """Per-core input shards."""
    m = {}
    x = np.asarray(inp["intra_attention_vectors"], np.float32)[:, ci * b:(ci + 1) * b, :]
    m["xT"] = np.ascontiguousarray(
        x.transpose(2, 0, 1).reshape(KI, 128, TB).transpose(1, 0, 2)).astype(NPBF16)
    st = np.asarray(inp["state_inter"], np.float32)[:, ci * b:(ci + 1) * b, :]
    for di, d in enumerate(("f", "b")):
        m["h0_" + d] = np.ascontiguousarray(
            st[di].T.reshape(KH, 128, b).transpose(1, 0, 2))
    return m


# ----------------------------------------------------------------- kernel build

def _build():
    nc = bacc.Bacc("TRN2", target_bir_lowering=False, debug=False)

    # inputs
    xT = nc.dram_tensor("xT", (128, KI, TB), BF16, kind="ExternalInput").ap()
    wih = {d: nc.dram_tensor(f"wih_{d}", (128, KI, MG, 128), BF16, kind="ExternalInput").ap()
           for d in ("f", "b")}
    whh = {d: nc.dram_tensor(f"whh_{d}", (128, KH, MG, 128), BF16, kind="ExternalInput").ap()
           for d in ("f", "b")}
    h0 = {d: nc.dram_tensor(f"h0_{d}", (128, KH, b), F32, kind="ExternalInput").ap()
          for d in ("f", "b")}
    xbias = {d: nc.dram_tensor(f"xbias_{d}", (128, MG), F32, kind="ExternalInput").ap()
             for d in ("f", "b")}
    hbias = {d: nc.dram_tensor(f"hbias_{d}", (KH, 128), BF16, kind="ExternalInput").ap()
             for d in ("f", "b")}
    onehot = nc.dram_tensor("onehot", (KH, KH * b), BF16, kind="ExternalInput").ap()
    winter = nc.dram_tensor("winter", (128, MJ, MJ, 128), F32R, kind="ExternalInput").ap()
    bias_inter = nc.dram_tensor("bias_inter", (128, MJ), F32, kind="ExternalInput").ap()
    wproj = nc.dram_tensor("wproj", (128, MJ), F32R, kind="ExternalInput").ap()
    wfinalT = nc.dram_tensor("wfinalT", (128, MJ, C), F32R, kind="ExternalInput").ap()
    bfinal = nc.dram_tensor("bfinal", (C, 1), F32, kind="ExternalInput").ap()

    # outputs
    o_final = nc.dram_tensor("o_final", (C, b), F32, kind="ExternalOutput").ap()
    o_state = {d: nc.dram_tensor(f"o_state_{d}", (128, KH, b), F32, kind="ExternalOutput").ap()
               for d in ("f", "b")}
    o_attn = nc.dram_tensor("o_attn", (1, TB), F32, kind="ExternalOutput").ap()

    Ident = mybir.ActivationFunctionType.Identity
    Sig = mybir.ActivationFunctionType.Sigmoid
    Tanh = mybir.ActivationFunctionType.Tanh

    with tile.TileContext(nc) as tc:
        with tc.tile_pool(name="dram", bufs=1, space="DRAM") as dpool:
            xgc = {d: [dpool.tile([128, MG, 512], F32, name=f"xg_{d}_{n}")
                       for n in range(NT)] for d in ("f", "b")}
            ybuf = {d: dpool.tile([128, KH, TB], F32, name=f"ybuf_{d}") for d in ("f", "b")}
            attn_d = dpool.tile([1, TB], F32, name="attn_d")

            # ---------------- Phase 1: xg = w_ihT.T @ xT (+ bias) ----------
            with tc.tile_pool(name="p1w", bufs=1) as p1w, \
                 tc.tile_pool(name="p1x", bufs=1) as p1x, \
                 tc.tile_pool(name="p1s", bufs=6) as p1s, \
                 tc.tile_pool(name="p1ps", bufs=8, space="PSUM") as p1ps:
                xt = p1x.tile([128, KI, TB], BF16, name="xt")
                wt, xb = {}, {}
                for d in ("f", "b"):
                    wt[d] = p1w.tile([128, KI, MG, 128], BF16, tag=f"wih_{d}", name=f"wih_{d}")
                    xb[d] = p1s.tile([128, MG], F32, tag=f"xbias_{d}", name=f"xb_{d}")
                    nc.sync.dma_start(out=xb[d][:], in_=xbias[d][:])
                # piecewise input loads, first-needed pieces first, so the
                # first matmuls start ~8us in instead of ~50us
                nc.sync.dma_start(out=xt[:, :, 0:512], in_=xT[:, :, 0:512])
                for k in range(KI):
                    nc.sync.dma_start(out=wt["f"][:, k], in_=wih["f"][:, k])
                nc.sync.dma_start(out=xt[:, :, 3 * 512:4 * 512], in_=xT[:, :, 3 * 512:4 * 512])
                for k in range(KI):
                    nc.sync.dma_start(out=wt["b"][:, k], in_=wih["b"][:, k])
                for n in (1, 2):
                    nc.sync.dma_start(out=xt[:, :, n * 512:(n + 1) * 512],
                                      in_=xT[:, :, n * 512:(n + 1) * 512])
                for d, n in [("f", 0), ("b", 3), ("f", 1), ("b", 2),
                             ("f", 2), ("b", 1), ("f", 3), ("b", 0)]:
                    for m in range(MG):
                        ps1 = p1ps.tile([128, 512], F32, tag="ps", name=f"ps_{d}_{n}_{m}")
                        for k in range(KI):
                            nc.tensor.matmul(
                                ps1[:], lhsT=wt[d][:, k, m, :],
                                rhs=xt[:, k, n * 512:(n + 1) * 512],
                                start=(k == 0), stop=(k == KI - 1))
                        stg = p1s.tile([128, 512], F32, tag="stage", name=f"stg_{d}_{n}_{m}")
                        nc.scalar.activation(stg[:], ps1[:], Ident, bias=xb[d][:, m:m + 1])
                        nc.sync.dma_start(out=xgc[d][n][:, m, :], in_=stg[:])

            # ---------------- Phase 2: GRU recurrences (fwd+bwd interleaved)
            with tc.tile_pool(name="p2w", bufs=1) as p2w, \
                 tc.tile_pool(name="p2st", bufs=1) as p2st, \
                 tc.tile_pool(name="p2xg", bufs=3) as p2xg, \
                 tc.tile_pool(name="p2y", bufs=2) as p2y, \
                 tc.tile_pool(name="p2g", bufs=2) as p2g, \
                 tc.tile_pool(name="p2ps", bufs=2, space="PSUM") as p2ps:
                wsb, hbf, hbmm, hm_ap = {}, {}, {}, {}
                oh = p2st.tile([KH, KH * b], BF16, tag="onehot", name="oh")
                nc.sync.dma_start(out=oh[:], in_=onehot[:])
                for d in ("f", "b"):
                    wsb[d] = p2w.tile([128, KH, MG, 128], BF16, tag=f"whh_{d}", name=f"whh_{d}")
                    nc.sync.dma_start(out=wsb[d][:], in_=whh[d][:])
                    h0t = p2st.tile([128, KH, b], F32, tag=f"h0_{d}", name=f"h0_{d}")
                    nc.sync.dma_start(out=h0t[:], in_=h0[d][:])
                    hbf[d] = p2st.tile([128, KH, b], BF16, tag=f"hbf_{d}", name=f"hbf_{d}")
                    nc.scalar.copy(hbf[d][:], h0t[:])
                    hbmm[d] = p2st.tile([KH, 128], BF16, tag=f"hb_{d}", name=f"hb_{d}")
                    nc.sync.dma_start(out=hbmm[d][:], in_=hbias[d][:])
                    hm_ap[d] = h0t[:]
                ones = p2st.tile([128, KH, b], F32, tag="ones", name="ones")
                nc.vector.memset(ones[:], 1.0)

                # pin static per-engine order (the scheduler's cost model
                # mis-orders the gate chains otherwise)
                _last = {"v": None, "s": None}

                def _ord(key, bi):
                    if _last[key] is not None:
                        add_dep_helper(bi.ins, _last[key], sync=False,
                                       reason="chain order")
                    _last[key] = bi.ins
                    return bi

                SPB = 8                      # steps per block
                NBLK = T // SPB              # 32
                for blk in range(NBLK):
                    xgt, yst, s0 = {}, {}, {}
                    for d in ("f", "b"):
                        # slot range covered by this block (contiguous, len SPB)
                        s0[d] = blk * SPB if d == "f" else T - SPB - blk * SPB
                        nch, off = divmod(s0[d] * b, 512)
                        xgt[d] = p2xg.tile([128, MG, SPB * b], F32, tag=f"xgb_{d}", name=f"xgb_{d}")
                        nc.sync.dma_start(out=xgt[d][:],
                                          in_=xgc[d][nch][:, :, off:off + SPB * b])
                        yst[d] = p2y.tile([128, KH, SPB * b], F32, tag=f"yst_{d}", name=f"yst_{d}")
                    for i in range(SPB):
                        for d in ("f", "b"):
                            loc = i if d == "f" else SPB - 1 - i
                            xs = xgt[d][:, :, loc * b:(loc + 1) * b]
                            psrz = p2ps.tile([128, 2 * KH, b], F32, tag=f"psrz_{d}", name=f"psrz_{d}")
                            psn = p2ps.tile([128, KH, b], F32, tag=f"psn_{d}", name=f"psn_{d}")
                            for mc in range(2 * KH):
                                for k in range(KH):
                                    nc.tensor.matmul(
                                        psrz[:, mc, :], lhsT=wsb[d][:, k, mc, :],
                                        rhs=hbf[d][:, k, :],
                                        start=(k == 0), stop=(k == KH - 1))
                            # bias lands in PSUM first via one-hot matmul
                            nc.tensor.matmul(
                                psn[:].rearrange("p c b -> p (c b)"),
                                lhsT=hbmm[d][:], rhs=oh[:],
                                start=True, stop=False)
                            for c in range(KH):
                                for k in range(KH):
                                    nc.tensor.matmul(
                                        psn[:, c, :], lhsT=wsb[d][:, k, 2 * KH + c, :],
                                        rhs=hbf[d][:, k, :],
                                        start=False, stop=(k == KH - 1 and c == KH - 1))
                            rz = p2g.tile([128, 2 * KH, b], F32, tag=f"rz_{d}", name=f"rz_{d}")
                            _ord("v", nc.vector.tensor_add(rz[:], xs[:, 0:2 * KH, :], psrz[:]))
                            _ord("s", nc.scalar.activation(rz[:], rz[:], Sig))
                            nb = p2g.tile([128, KH, b], F32, tag=f"nb_{d}", name=f"nb_{d}")
                            _ord("v", nc.vector.scalar_tensor_tensor(
                                out=nb[:], in0=psn[:], scalar=0.0, in1=rz[:, 0:KH, :],
                                op0=mybir.AluOpType.add, op1=mybir.AluOpType.mult))
                            _ord("v", nc.vector.tensor_add(nb[:], nb[:], xs[:, 2 * KH:3 * KH, :]))
                            _ord("s", nc.scalar.activation(nb[:], nb[:], Tanh))
                            # u = 1-z, hz = h*z: off the tail, run during tanh
                            u = p2g.tile([128, KH, b], F32, tag=f"u_{d}", name=f"u_{d}")
                            _ord("v", nc.vector.scalar_tensor_tensor(
                                out=u[:], in0=rz[:, KH:2 * KH, :], scalar=-1.0, in1=ones[:],
                                op0=mybir.AluOpType.mult, op1=mybir.AluOpType.add))
                            hz = p2g.tile([128, KH, b], F32, tag=f"hz_{d}", name=f"hz_{d}")
                            _ord("v", nc.vector.tensor_mul(hz[:], hm_ap[d], rz[:, KH:2 * KH, :]))
                            _ord("v", nc.vector.tensor_mul(nb[:], nb[:], u[:]))
                            # critical: bf16 h for next step's matmuls
                            _ord("v", nc.vector.tensor_add(hbf[d][:], nb[:], hz[:]))
                            hnew = yst[d][:, :, loc * b:(loc + 1) * b]
                            nc.gpsimd.tensor_add(hnew, nb[:], hz[:])
                            hm_ap[d] = hnew
                    for d in ("f", "b"):
                        nc.sync.dma_start(
                            out=ybuf[d][:, :, s0[d] * b:(s0[d] + SPB) * b], in_=yst[d][:])
                for d in ("f", "b"):
                    nc.sync.dma_start(out=o_state[d][:], in_=hm_ap[d])

            # ---------------- Phase 3: attention pooling -------------------
            with tc.tile_pool(name="p3w", bufs=1) as p3w, \
                 tc.tile_pool(name="p3y", bufs=2) as p3y, \
                 tc.tile_pool(name="p3s", bufs=2) as p3s, \
                 tc.tile_pool(name="p3a", bufs=2) as p3a, \
                 tc.tile_pool(name="p3ps", bufs=2, space="PSUM") as p3ps:
                wi = p3w.tile([128, MJ, MJ, 128], F32R)
                nc.sync.dma_start(out=wi[:], in_=winter[:])
                bi = p3w.tile([128, MJ], F32)
                nc.sync.dma_start(out=bi[:], in_=bias_inter[:])
                wp = p3w.tile([128, MJ], F32R)
                nc.sync.dma_start(out=wp[:], in_=wproj[:])
                wf = p3w.tile([128, MJ, C], F32R)
                nc.sync.dma_start(out=wf[:], in_=wfinalT[:])
                bf_sb = p3w.tile([C, 1], F32)
                nc.sync.dma_start(out=bf_sb[:], in_=bfinal[:])
                acc = p3w.tile([C, b], F32)
                nc.vector.memset(acc[:], 0.0)

                for n in range(NT):
                    sl = slice(n * 512, (n + 1) * 512)
                    yc = {}
                    for d in ("f", "b"):
                        yc[d] = p3y.tile([128, KH, 512], F32R, tag=f"y_{d}", name=f"yc_{d}")
                        nc.sync.dma_start(out=yc[d][:], in_=ybuf[d][:, :, sl].bitcast(F32R))

                    def rhs_chunk(k):
                        d = "f" if k < KH else "b"
                        return yc[d][:, k % KH, :]

                    sq = p3s.tile([128, MJ, 512], F32R, tag="sq")
                    for m in range(MJ):
                        psq = p3ps.tile([128, 512], F32, tag="psq")
                        for k in range(MJ):
                            nc.tensor.matmul(psq[:], lhsT=wi[:, k, m, :],
                                             rhs=rhs_chunk(k),
                                             start=(k == 0), stop=(k == MJ - 1))
                        nc.scalar.activation(sq[:, m, :], psq[:], Tanh,
                                             bias=bi[:, m:m + 1])
                    psa = p3ps.tile([1, 512], F32, tag="psa")
                    for k in range(MJ):
                        nc.tensor.matmul(psa[:], lhsT=wp[:, k:k + 1],
                                         rhs=sq[:, k, :],
                                         start=(k == 0), stop=(k == MJ - 1))
                    psp = p3ps.tile([C, 512], F32, tag="psp")
                    for k in range(MJ):
                        nc.tensor.matmul(psp[:], lhsT=wf[:, k, :],
                                         rhs=rhs_chunk(k),
                                         start=(k == 0), stop=(k == MJ - 1))
                    asb = p3a.tile([1, 512], F32, tag="asb")
                    nc.vector.tensor_copy(asb[:], psa[:])
                    nc.sync.dma_start(out=o_attn[:, sl], in_=asb[:])
                    nc.sync.dma_start(out=attn_d[:, sl], in_=asb[:])
                    abc = p3a.tile([C, 512], F32, tag="abc")
                    asl = attn_d[0:1, sl]
                    bc_ap = bass.AP(tensor=asl.tensor, offset=asl.offset,
                                    ap=[[0, C]] + list(asl.ap)[1:])
                    nc.gpsimd.dma_start(out=abc[:], in_=bc_ap)
                    pw = p3a.tile([C, 512], F32, tag="pw")
                    nc.vector.tensor_mul(pw[:], psp[:], abc[:])
                    part = p3a.tile([C, b], F32, tag="part")
                    nc.vector.tensor_reduce(
                        part[:], pw[:].rearrange("p (t bb) -> p bb t", bb=b),
                        axis=mybir.AxisListType.X, op=mybir.AluOpType.add)
                    nc.vector.tensor_add(acc[:], acc[:], part[:])

                ofin = p3a.tile([C, b], F32, tag="ofin")
                nc.vector.tensor_scalar_add(ofin[:], acc[:], bf_sb[:, 0:1])
                nc.sync.dma_start(out=o_final[:], in_=ofin[:])

    nc.compile()
    return nc


# ----------------------------------------------------------------- entry point

def kernel(**inputs):
    if "nc" not in _CACHE:
        _CACHE["nc"] = _build()
    nc = _CACHE["nc"]

    shared = _prep_shared(inputs)
    in_maps = []
    for ci in range(NCORES):
        m = dict(shared)
        m.update(_prep_core(inputs, ci))
        in_maps.append(m)

    res = bass_utils.run_bass_kernel_spmd(
        nc, in_maps, core_ids=list(range(NCORES)), trace=TRACE)
    _CACHE["last_result"] = res

    finals, states_f, states_b, attns = [], [], [], []
    for ci in range(NCORES):
        out = res.results[ci]
        finals.append(np.asarray(out["o_final"]).T)                     # [b, C]
        states_f.append(np.asarray(out["o_state_f"]).transpose(2, 1, 0).reshape(b, H))
        states_b.append(np.asarray(out["o_state_b"]).transpose(2, 1, 0).reshape(b, H))
        attns.append(np.asarray(out["o_attn"]).reshape(T, b).T)         # [b, T]
    final_map = np.concatenate(finals, 0).astype(np.float32)
    state_out = np.stack([np.concatenate(states_f, 0),
                          np.concatenate(states_b, 0)], 0).astype(np.float32)
    attn_T = np.concatenate(attns, 0).astype(np.float32)
    return final_map, state_out, attn_T
